# revision 22
# baseline (speedup 1.0000x reference)
"""Trainium2 Bass kernel for agent attention (sparse_attention problem).

Per-core work (data-parallel over batch B=8 across 8 NeuronCores):
  x[b] [256, 64, 64] -> qkv 3x3 conv (dif-conv + BN folded into weights)
  -> agent attention (8 heads, d=32, 64 agent tokens)
  -> depthwise 3x3 pe conv on v -> 1x1 proj.

v3: f32r conv matmuls (self-loading weights, no exposed LDWEIGHTS),
bf16 input staging for fast DMA, GpSimd depthwise pe conv, paired
stage-1 exps, restructured stage-2 (g = attnZ^T @ e2 full-width
matmuls, matmul-broadcast softmax denominator, fast reciprocal).
"""
import numpy as np

NUM_HEADS = 8
AGENT_NUM = 64
THETA = 0.7
C = 256
H = W = 64
HW = H * W
D = C // NUM_HEADS          # 32
N_AG = AGENT_NUM            # 64
PS = 8                      # pool size
N_CORES = 8
B = 8

_cache = {}


def _build():
    import concourse.bass as bass
    import concourse.tile as tile
    from concourse import bacc, mybir

    f32 = mybir.dt.float32
    f32r = mybir.dt.float32r
    bf16 = mybir.dt.bfloat16
    AF = mybir.ActivationFunctionType
    ALU = mybir.AluOpType
    AX = mybir.AxisListType

    nc = bacc.Bacc("TRN2", target_bir_lowering=False, debug=False,
                   enable_asserts=True, num_devices=N_CORES)

    X = nc.dram_tensor("x", [2, 128, H, W], bf16, kind="ExternalInput").ap()
    WQ = nc.dram_tensor("wq", [6, 128, 2, 9, 128], f32r,
                        kind="ExternalInput").ap()
    BQ = nc.dram_tensor("bq", [128, 6], f32, kind="ExternalInput").ap()
    PEW = nc.dram_tensor("pew", [128, 2, 9], f32, kind="ExternalInput").ap()
    PW = nc.dram_tensor("pw", [128, 2 * 256], f32r, kind="ExternalInput").ap()
    PB = nc.dram_tensor("pb", [128, 2], f32, kind="ExternalInput").ap()
    IDN = nc.dram_tensor("idn", [128, 128], bf16, kind="ExternalInput").ap()
    OUT = nc.dram_tensor("out", [2, 128, HW], f32, kind="ExternalOutput").ap()

    # softmax exp scale: d^-0.5, with the 1/64 agent-pool mean folded in
    SCALE = (D ** -0.5) / (PS * PS)

    with tile.TileContext(nc) as tc:
        from contextlib import ExitStack
        with ExitStack() as top:
            pers = top.enter_context(tc.tile_pool(name="pers", bufs=1))
            q_sb = [pers.tile([128, HW], f32r, tag=f"q{i}", name=f"q{i}")
                    for i in range(2)]
            k_sb = [pers.tile([128, HW], bf16, tag=f"k{i}", name=f"k{i}")
                    for i in range(2)]
            v_pad = [pers.tile([128, 66 * 66], bf16, tag=f"vp{i}",
                               name=f"vp{i}") for i in range(2)]
            att_out = [pers.tile([128, HW], f32r, tag=f"ao{i}", name=f"ao{i}")
                       for i in range(2)]
            bq = pers.tile([128, 6], f32, tag="bq", name="bq")
            idn = pers.tile([128, 128], bf16, tag="idn", name="idn")
            pew = pers.tile([128, 2, 9], f32, tag="pew", name="pew")
            asum_t = pers.tile([128, 128], f32, tag="asum", name="asum")
            a_sum = [asum_t[:, 64 * i:64 * (i + 1)] for i in range(2)]
            # a_bd4 needed in bf16 (stage-1 rhs) and f32r (stage-2 lhsT)
            abd_bf_t = pers.tile([128, 512], bf16, tag="abdb", name="abdb")
            abd_bf = [abd_bf_t[:, 256 * i:256 * (i + 1)] for i in range(2)]
            abd_f_t = pers.tile([128, 512], f32r, tag="abdf", name="abdf")
            az_t = pers.tile([128, 4 * 64], bf16, tag="az", name="az")
            attnZ = [az_t[:, 64 * i:64 * (i + 1)] for i in range(4)]
            pw = pers.tile([128, 2 * 256], f32r, tag="pw", name="pwt")
            pb = pers.tile([128, 2], f32, tag="pb", name="pbt")
            hsel = pers.tile([128, 64], bf16, tag="hsel", name="hsel")
            nc.sync.dma_start(bq[:], BQ[:])
            nc.sync.dma_start(idn[:], IDN[:])
            nc.sync.dma_start(pew[:], PEW[:])
            nc.sync.dma_start(pw[:], PW[:])
            nc.sync.dma_start(pb[:], PB[:])
            pwv = pw[:].rearrange("p (a b) -> p a b", a=2, b=256)

            # hsel: block-ones selector so hsel^T @ e2 replicates each
            # head's agent-sum across that head's 32 d-partitions
            nc.vector.memset(hsel[:], 0.0)
            nc.vector.memset(hsel[0:64, 0:32], 1.0)
            nc.vector.memset(hsel[64:128, 32:64], 1.0)

            for cc in range(2):
                vv = v_pad[cc][:].rearrange("p (r c) -> p r c", r=66, c=66)
                nc.vector.memset(vv[:, 0:1, :], 0.0)
                nc.vector.memset(vv[:, 65:66, :], 0.0)
                nc.vector.memset(vv[:, :, 0:1], 0.0)
                nc.vector.memset(vv[:, :, 65:66], 0.0)

            # persistent v^T chunk tiles (layout per cc-section of 130:
            # [64 ch of hp-even | 2 ones | 64 ch of hp-odd])
            s1sb = top.enter_context(tc.tile_pool(name="s1sb", bufs=1))
            vts = [s1sb.tile([128, 260], bf16, tag=f"vt{i}", name=f"vt{i}")
                   for i in range(32)]
            for i in range(32):
                for cc in range(2):
                    nc.vector.memset(vts[i][:, 130 * cc + 64:130 * cc + 66],
                                     1.0)

            with ExitStack() as ph:
                tr_ps = ph.enter_context(
                    tc.tile_pool(name="trps", bufs=2, space="PSUM"))
                cpool = ph.enter_context(tc.tile_pool(name="conv", bufs=1))
                wpool = ph.enter_context(tc.tile_pool(name="wq", bufs=1))
                cps = ph.enter_context(
                    tc.tile_pool(name="cps", bufs=4, space="PSUM"))

                x_pad = [cpool.tile([128, 66 * 66], f32r, tag=f"xp{i}",
                                    name=f"xp{i}") for i in range(2)]
                x_stg = [cpool.tile([128, HW], bf16, tag=f"xs{i}",
                                    name=f"xs{i}") for i in range(2)]
                for kc in range(2):
                    xv = x_pad[kc][:].bitcast(f32).rearrange(
                        "p (r c) -> p r c", r=66, c=66)
                    nc.vector.memset(xv[:, 0:1, :], 0.0)
                    nc.vector.memset(xv[:, 65:66, :], 0.0)
                    nc.vector.memset(xv[:, :, 0:1], 0.0)
                    nc.vector.memset(xv[:, :, 65:66], 0.0)

                # first conv group's weights ahead of the bulk x transfer;
                # x lands as one fully-contiguous transfer per group and
                # DVE pads/casts it to f32
                wq4 = []
                for kc in range(2):
                    wt = wpool.tile([128, 9, 128], f32r, tag="w", name="w",
                                    bufs=4)
                    nc.sync.dma_start(wt[:], WQ[4, :, kc])
                    wq4.append(wt)
                for kc in range(2):
                    nc.scalar.dma_start(x_stg[kc][:],
                                        X[kc].rearrange("p r c -> p (r c)"))
                for r0 in range(0, 64, 16):
                    for kc in range(2):
                        xv = x_pad[kc][:].rearrange(
                            "p (r c) -> p r c", r=66, c=66)
                        xs = x_stg[kc][:].rearrange("p (r c) -> p r c",
                                                    r=64, c=64)
                        nc.vector.tensor_copy(
                            xv[:, r0 + 1:r0 + 17, 1:65],
                            xs[:, r0:r0 + 16, :])

                def conv_group(mc, wts=None, extra=None):
                    if wts is None:
                        wts = []
                        for kc in range(2):
                            wt = wpool.tile([128, 9, 128], f32r, tag="w",
                                            name="w", bufs=4)
                            nc.sync.dma_start(wt[:], WQ[mc, :, kc])
                            wts.append(wt)
                    for rb in range(8):
                        ps_t = cps.tile([128, 512], f32, tag="cps",
                                        name="cpst")
                        psv = ps_t[:].rearrange("p (r c) -> p r c", r=8, c=64)
                        i = 0
                        for kc in range(2):
                            xv = x_pad[kc][:].rearrange(
                                "p (r c) -> p r c", r=66, c=66)
                            for s in range(9):
                                ky, kx = s // 3, s % 3
                                rhs = xv[:, 8 * rb + ky: 8 * rb + ky + 8,
                                         kx: kx + 64]
                                nc.tensor.matmul(
                                    psv, wts[kc][:, s, :], rhs,
                                    start=(i == 0), stop=(i == 17))
                                i += 1
                        bias = bq[:, mc: mc + 1]
                        if mc < 2:
                            dst = q_sb[mc][:, 512 * rb: 512 * (rb + 1)]
                            nc.scalar.add(dst, ps_t[:], bias)
                        elif mc < 4:
                            dst = k_sb[mc - 2][:, 512 * rb: 512 * (rb + 1)]
                            nc.scalar.add(dst, ps_t[:], bias)
                        else:
                            vv = v_pad[mc - 4][:].rearrange(
                                "p (r c) -> p r c", r=66, c=66)
                            dst = vv[:, 8 * rb + 1: 8 * rb + 9, 1:65]
                            nc.scalar.add(dst, psv, bias)
                        if extra is not None:
                            extra(rb)

                # transposed v chunk builder: DVE staging copy + one PE
                # transpose + 2 DVE copies per (ch, cc)
                def make_vt(ch):
                    vtc = vts[ch]
                    for cc in range(2):
                        vv = v_pad[cc][:].rearrange(
                            "p (r c) -> p r c", r=66, c=66)
                        vstg = wpool.tile([128, 128], bf16, tag="vstg",
                                          name="vstg", bufs=4)
                        nc.vector.tensor_copy(
                            vstg[:].rearrange("p (r c) -> p r c", r=2, c=64),
                            vv[:, 2 * ch + 1: 2 * ch + 3, 1:65])
                        tp = tr_ps.tile([128, 128], bf16, tag="tr",
                                        name="trt")
                        nc.tensor.transpose(tp[:], vstg[:], idn[:])
                        nc.vector.tensor_copy(
                            vtc[:, 130 * cc:130 * cc + 64], tp[:, 0:64])
                        nc.vector.tensor_copy(
                            vtc[:, 130 * cc + 66:130 * cc + 130],
                            tp[:, 64:128])

                # pe depthwise conv on DVE, accumulating into att_out
                def pe_conv(cc, g):
                    vvf = v_pad[cc][:].rearrange(
                        "p (r c) -> p r c", r=66, c=66)
                    aof = att_out[cc][:].rearrange(
                        "p (r c) -> p r c", r=64, c=64)
                    r0 = 16 * g
                    dst = aof[:, r0:r0 + 16, :]
                    for s in range(9):
                        ky, kx = s // 3, s % 3
                        sv = vvf[:, r0 + ky: r0 + ky + 16, kx: kx + 64]
                        if s == 0:
                            nc.vector.tensor_scalar_mul(
                                dst, sv, pew[:, cc, 0:1])
                        else:
                            nc.vector.scalar_tensor_tensor(
                                dst, sv, pew[:, cc, s:s + 1], dst,
                                ALU.mult, ALU.add)

                # v first
                conv_group(4, wts=wq4)
                conv_group(5)

                # during q/k conv: transposes spread 1 per rb, pe on GpSimd
                nvt = [0]

                def vt_extra(rb):
                    if nvt[0] < 32:
                        make_vt(nvt[0])
                        nvt[0] += 1

                conv_group(0, extra=vt_extra)
                pe_conv(0, 0)
                pe_conv(0, 1)
                conv_group(1, extra=vt_extra)
                pe_conv(0, 2)
                pe_conv(0, 3)

                # pooling + block-diag a (both dtypes)
                for ccq in range(2):
                    qv = q_sb[ccq][:].rearrange(
                        "p (by dy bx dx) -> p by bx dy dx",
                        by=8, dy=8, bx=8, dx=8)
                    nc.vector.tensor_reduce(a_sum[ccq], qv, AX.XY, ALU.add)
                nc.vector.memset(abd_bf_t[:], 0.0)
                nc.vector.memset(abd_f_t[:].bitcast(f32), 0.0)
                for cc in range(2):
                    for j in range(4):
                        nc.vector.tensor_copy(
                            abd_bf[cc][32 * j:32 * j + 32,
                                       64 * j:64 * j + 64],
                            a_sum[cc][32 * j:32 * j + 32, :])
                        nc.vector.tensor_copy(
                            abd_f_t[32 * j:32 * j + 32,
                                    256 * cc + 64 * j:256 * cc + 64 * j + 64],
                            a_sum[cc][32 * j:32 * j + 32, :])

                # k
                conv_group(2, extra=vt_extra)
                pe_conv(1, 0)
                pe_conv(1, 1)
                conv_group(3, extra=vt_extra)
                pe_conv(1, 2)
                pe_conv(1, 3)
                while nvt[0] < 32:
                    make_vt(nvt[0])
                    nvt[0] += 1

            # ---- stage 1 ----
            # attn_ps[hp] accumulates [128 agents, 66] over 32 chunks:
            # for half 0 cols = [64 ch | Z Z], for half 1 cols = [Z Z | 64 ch]
            with ExitStack() as ph:
                st_ps = ph.enter_context(
                    tc.tile_pool(name="stps", bufs=3, space="PSUM"))
                at_ps = ph.enter_context(
                    tc.tile_pool(name="atps", bufs=4, space="PSUM"))
                etp = ph.enter_context(tc.tile_pool(name="etp", bufs=1))
                attn_ps = [at_ps.tile([128, 66], f32, tag="at", name="at")
                           for _ in range(4)]
                for chp in range(16):   # ch pairs
                    for cc in range(2):
                        sp = st_ps.tile([128, 512], f32, tag="st", name="stt")
                        for u in range(2):
                            ch = 2 * chp + u
                            nc.tensor.matmul(
                                sp[:, 256 * u:256 * (u + 1)],
                                k_sb[cc][:, 128 * ch:128 * (ch + 1)],
                                abd_bf[cc], start=True, stop=True,
                                skip_group_check=True)
                        et = etp.tile([128, 512], bf16, tag="et", name="et",
                                      bufs=4)
                        nc.scalar.activation(et[:], sp[:], AF.Exp, scale=SCALE)
                        for u in range(2):
                            ch = 2 * chp + u
                            for half in range(2):
                                hp = 2 * cc + half
                                rhs = vts[ch][:, 130 * cc + 64 * half:
                                              130 * cc + 64 * half + 66]
                                nc.tensor.matmul(
                                    attn_ps[hp][:],
                                    et[:, 256 * u + 128 * half:
                                       256 * u + 128 * (half + 1)],
                                    rhs, start=(ch == 0), stop=(ch == 31))

                # normalize stage-1 rows by Z1 -> attnZ [128 agents, 64]
                for hp in range(4):
                    half = hp % 2
                    zc = 64 if half == 0 else 0
                    och = 0 if half == 0 else 2
                    r1 = etp.tile([128, 1], f32, tag="r1", name="r1", bufs=4)
                    nc.vector.reciprocal(r1[:], attn_ps[hp][:, zc:zc + 1])
                    nc.vector.memset(attnZ[hp], 0.0)
                    nc.vector.tensor_scalar_mul(
                        attnZ[hp][0:64, 0:32],
                        attn_ps[hp][0:64, och:och + 32], r1[0:64, :])
                    nc.vector.tensor_scalar_mul(
                        attnZ[hp][64:128, 32:64],
                        attn_ps[hp][64:128, och + 32:och + 64],
                        r1[64:128, :])

            # ---- stage 2 + proj ----
            with ExitStack() as ph:
                s2sb = ph.enter_context(tc.tile_pool(name="s2sb", bufs=4))
                osb = ph.enter_context(tc.tile_pool(name="osb", bufs=3))
                s2_ps = ph.enter_context(
                    tc.tile_pool(name="s2ps", bufs=2, space="PSUM"))
                g_ps = ph.enter_context(
                    tc.tile_pool(name="gps", bufs=2, space="PSUM"))
                z_ps = ph.enter_context(
                    tc.tile_pool(name="zps", bufs=2, space="PSUM"))
                pr_ps = ph.enter_context(
                    tc.tile_pool(name="prps", bufs=2, space="PSUM"))

                for nt in range(8):
                    for cc in range(2):
                        gp = g_ps.tile([128, 512], f32, tag="g", name="gt")
                        zp = z_ps.tile([128, 512], f32, tag="z", name="zt")
                        for half in range(2):
                            hp = 2 * cc + half
                            sp = s2_ps.tile([128, 512], f32, tag="s2",
                                            name="s2t")
                            nc.tensor.matmul(
                                sp[:],
                                abd_f_t[:, 256 * cc + 128 * half:
                                        256 * cc + 128 * (half + 1)],
                                q_sb[cc][:, 512 * nt:512 * (nt + 1)],
                                start=True, stop=True)
                            e2 = s2sb.tile([128, 512], bf16, tag="e2",
                                           name="e2")
                            nc.scalar.activation(e2[:], sp[:], AF.Exp,
                                                 scale=SCALE)
                            # g rows 0:64 (half 0) / 64:128 (half 1)
                            nc.tensor.matmul(
                                gp[64 * half:64 * half + 64, :],
                                attnZ[hp], e2[:], start=True, stop=True,
                                skip_group_check=True)
                            # Zb rows: per-head agent sums of e2, already
                            # replicated to each head's 32 d-partitions
                            nc.tensor.matmul(
                                zp[64 * half:64 * half + 64, :],
                                hsel[:], e2[:], start=True, stop=True,
                                skip_group_check=True)
                        rb = s2sb.tile([128, 512], f32, tag="rb", name="rbt")
                        nc.vector.reciprocal_approx_fast(rb[:], zp[:])
                        tsc = s2sb.tile([128, 512], f32r, tag="ts",
                                        name="tsc")
                        nc.vector.tensor_tensor(tsc[:], gp[:], rb[:],
                                                ALU.mult)
                        sl = att_out[cc][:, 512 * nt:512 * (nt + 1)]
                        nc.gpsimd.tensor_tensor(sl, tsc[:].bitcast(f32),
                                                sl.bitcast(f32), ALU.add)
                    for mc in range(2):
                        pp = pr_ps.tile([128, 512], f32, tag="tp", name="prt")
                        for kc in range(2):
                            nc.tensor.matmul(
                                pp[:], pwv[:, kc, 128 * mc:128 * (mc + 1)],
                                att_out[kc][:, 512 * nt:512 * (nt + 1)],
                                start=(kc == 0), stop=(kc == 1))
                        ot = osb.tile([128, 512], f32, tag="ot", name="ott")
                        nc.vector.tensor_scalar_add(ot[:], pp[:],
                                                    pb[:, mc:mc + 1])
                        nc.sync.dma_start(
                            OUT[mc, :, 512 * nt:512 * (nt + 1)], ot[:])

    nc.compile()
    return nc


def _prep_consts(qkv_w, qkv_s, qkv_b, pe_w, pe_s, pe_b, proj_w, proj_s,
                 proj_b):
    import ml_dtypes
    f = np.float32
    bf = ml_dtypes.bfloat16
    w = np.asarray(qkv_w, f).copy()          # [768, 256, 3, 3]
    dif = (w[:, :, 0, 1] + w[:, :, 1, 0] + w[:, :, 1, 1] + w[:, :, 1, 2]
           + w[:, :, 2, 1])
    w[:, :, 1, 1] -= THETA * dif
    w *= np.asarray(qkv_s, f)[:, None, None, None]
    # WQ[mc, p, kc, s, o'] = w[128*mc+o', 128*kc+p, s//3, s%3]
    wq = w.reshape(6, 128, 2, 128, 9)        # [mc, o', kc, p, s]
    wq = np.ascontiguousarray(wq.transpose(0, 3, 2, 4, 1))  # [6,128,2,9,128]

    bq = np.ascontiguousarray(np.asarray(qkv_b, f).reshape(6, 128).T)

    pe_wf = np.asarray(pe_w, f)[:, 0] * np.asarray(pe_s, f)[:, None, None]
    pew = np.zeros((128, 2, 9), f)
    for kc in range(2):
        for s in range(9):
            pew[:, kc, s] = pe_wf[128 * kc:128 * (kc + 1), s // 3, s % 3]

    pwm = np.asarray(proj_w, f)[:, :, 0, 0] * np.asarray(proj_s, f)[:, None]
    pw = np.ascontiguousarray(
        pwm.T.reshape(2, 128, 256).transpose(1, 0, 2).reshape(128, 512))
    pbv = np.asarray(proj_b, f) + pwm @ np.asarray(pe_b, f)
    pb = np.ascontiguousarray(pbv.reshape(2, 128).T)

    idn = np.eye(128, dtype=bf)
    return dict(wq=wq, bq=bq, pew=pew, pw=pw, pb=pb, idn=idn)


def kernel(x, qkv_w, qkv_s, qkv_b, pe_w, pe_s, pe_b, proj_w, proj_s, proj_b):
    import ml_dtypes
    from concourse.bass_utils import run_bass_kernel_spmd

    if "nc" not in _cache:
        _cache["nc"] = _build()
    nc = _cache["nc"]

    consts = _prep_consts(qkv_w, qkv_s, qkv_b, pe_w, pe_s, pe_b, proj_w,
                          proj_s, proj_b)
    x = np.asarray(x, np.float32).astype(ml_dtypes.bfloat16)
    in_maps = []
    for b in range(B):
        m = dict(consts)
        m["x"] = np.ascontiguousarray(x[b].reshape(2, 128, H, W))
        in_maps.append(m)

    res = run_bass_kernel_spmd(nc, in_maps, list(range(N_CORES)), trace=False)
    out = np.empty((B, C, H, W), np.float32)
    for b in range(B):
        out[b] = res.results[b]["out"].reshape(C, H, W)
    return out


# revision 23
# speedup vs baseline: 1.0047x; 1.0047x over previous
"""Trainium2 Bass kernel for agent attention (sparse_attention problem).

Per-core work (data-parallel over batch B=8 across 8 NeuronCores):
  x[b] [256, 64, 64] -> qkv 3x3 conv (dif-conv + BN folded into weights)
  -> agent attention (8 heads, d=32, 64 agent tokens)
  -> depthwise 3x3 pe conv on v -> 1x1 proj.

v3: f32r conv matmuls (self-loading weights, no exposed LDWEIGHTS),
bf16 input staging for fast DMA, GpSimd depthwise pe conv, paired
stage-1 exps, restructured stage-2 (g = attnZ^T @ e2 full-width
matmuls, matmul-broadcast softmax denominator, fast reciprocal).
"""
import numpy as np

NUM_HEADS = 8
AGENT_NUM = 64
THETA = 0.7
C = 256
H = W = 64
HW = H * W
D = C // NUM_HEADS          # 32
N_AG = AGENT_NUM            # 64
PS = 8                      # pool size
N_CORES = 8
B = 8

_cache = {}


def _build():
    import concourse.bass as bass
    import concourse.tile as tile
    from concourse import bacc, mybir

    f32 = mybir.dt.float32
    f32r = mybir.dt.float32r
    bf16 = mybir.dt.bfloat16
    AF = mybir.ActivationFunctionType
    ALU = mybir.AluOpType
    AX = mybir.AxisListType

    nc = bacc.Bacc("TRN2", target_bir_lowering=False, debug=False,
                   enable_asserts=True, num_devices=N_CORES)

    X = nc.dram_tensor("x", [2, 128, H, W], bf16, kind="ExternalInput").ap()
    WQ = nc.dram_tensor("wq", [6, 128, 2, 9, 128], f32r,
                        kind="ExternalInput").ap()
    BQ = nc.dram_tensor("bq", [128, 6], f32, kind="ExternalInput").ap()
    PEW = nc.dram_tensor("pew", [128, 2, 9], f32, kind="ExternalInput").ap()
    PW = nc.dram_tensor("pw", [128, 2 * 256], f32r, kind="ExternalInput").ap()
    PB = nc.dram_tensor("pb", [128, 2], f32, kind="ExternalInput").ap()
    IDN = nc.dram_tensor("idn", [128, 128], bf16, kind="ExternalInput").ap()
    OUT = nc.dram_tensor("out", [2, 128, HW], f32, kind="ExternalOutput").ap()

    # softmax exp scale: d^-0.5, with the 1/64 agent-pool mean folded in
    SCALE = (D ** -0.5) / (PS * PS)

    with tile.TileContext(nc) as tc:
        from contextlib import ExitStack
        with ExitStack() as top:
            pers = top.enter_context(tc.tile_pool(name="pers", bufs=1))
            q_sb = [pers.tile([128, HW], f32r, tag=f"q{i}", name=f"q{i}")
                    for i in range(2)]
            k_sb = [pers.tile([128, HW], bf16, tag=f"k{i}", name=f"k{i}")
                    for i in range(2)]
            v_pad = [pers.tile([128, 66 * 66], bf16, tag=f"vp{i}",
                               name=f"vp{i}") for i in range(2)]
            att_out = [pers.tile([128, HW], f32r, tag=f"ao{i}", name=f"ao{i}")
                       for i in range(2)]
            bq = pers.tile([128, 6], f32, tag="bq", name="bq")
            idn = pers.tile([128, 128], bf16, tag="idn", name="idn")
            pew = pers.tile([128, 2, 9], f32, tag="pew", name="pew")
            asum_t = pers.tile([128, 128], f32, tag="asum", name="asum")
            a_sum = [asum_t[:, 64 * i:64 * (i + 1)] for i in range(2)]
            # a_bd4 needed in bf16 (stage-1 rhs) and f32r (stage-2 lhsT)
            abd_bf_t = pers.tile([128, 512], bf16, tag="abdb", name="abdb")
            abd_bf = [abd_bf_t[:, 256 * i:256 * (i + 1)] for i in range(2)]
            abd_f_t = pers.tile([128, 512], f32r, tag="abdf", name="abdf")
            az_t = pers.tile([128, 4 * 64], bf16, tag="az", name="az")
            attnZ = [az_t[:, 64 * i:64 * (i + 1)] for i in range(4)]
            pw = pers.tile([128, 2 * 256], f32r, tag="pw", name="pwt")
            pb = pers.tile([128, 2], f32, tag="pb", name="pbt")
            hsel = pers.tile([128, 64], bf16, tag="hsel", name="hsel")
            nc.sync.dma_start(bq[:], BQ[:])
            nc.sync.dma_start(idn[:], IDN[:])
            nc.sync.dma_start(pew[:], PEW[:])
            nc.sync.dma_start(pw[:], PW[:])
            nc.sync.dma_start(pb[:], PB[:])
            pwv = pw[:].rearrange("p (a b) -> p a b", a=2, b=256)

            # hsel: block-ones selector so hsel^T @ e2 replicates each
            # head's agent-sum across that head's 32 d-partitions
            nc.vector.memset(hsel[:], 0.0)
            nc.vector.memset(hsel[0:64, 0:32], 1.0)
            nc.vector.memset(hsel[64:128, 32:64], 1.0)

            for cc in range(2):
                vv = v_pad[cc][:].rearrange("p (r c) -> p r c", r=66, c=66)
                nc.vector.memset(vv[:, 0:1, :], 0.0)
                nc.vector.memset(vv[:, 65:66, :], 0.0)
                nc.vector.memset(vv[:, :, 0:1], 0.0)
                nc.vector.memset(vv[:, :, 65:66], 0.0)

            # persistent v^T chunk tiles (layout per cc-section of 130:
            # [64 ch of hp-even | 2 ones | 64 ch of hp-odd])
            s1sb = top.enter_context(tc.tile_pool(name="s1sb", bufs=1))
            vts = [s1sb.tile([128, 260], bf16, tag=f"vt{i}", name=f"vt{i}")
                   for i in range(32)]

            with ExitStack() as ph:
                tr_ps = ph.enter_context(
                    tc.tile_pool(name="trps", bufs=2, space="PSUM"))
                cpool = ph.enter_context(tc.tile_pool(name="conv", bufs=1))
                wpool = ph.enter_context(tc.tile_pool(name="wq", bufs=1))
                cps = ph.enter_context(
                    tc.tile_pool(name="cps", bufs=4, space="PSUM"))

                x_pad = [cpool.tile([128, 66 * 66], f32r, tag=f"xp{i}",
                                    name=f"xp{i}") for i in range(2)]
                x_stg = [cpool.tile([128, HW], bf16, tag=f"xs{i}",
                                    name=f"xs{i}") for i in range(2)]
                for kc in range(2):
                    xv = x_pad[kc][:].bitcast(f32).rearrange(
                        "p (r c) -> p r c", r=66, c=66)
                    nc.vector.memset(xv[:, 0:1, :], 0.0)
                    nc.vector.memset(xv[:, 65:66, :], 0.0)
                    nc.vector.memset(xv[:, :, 0:1], 0.0)
                    nc.vector.memset(xv[:, :, 65:66], 0.0)

                # first conv group's weights ahead of the bulk x transfer;
                # x lands as one fully-contiguous transfer per group and
                # DVE pads/casts it to f32
                wq4 = []
                for kc in range(2):
                    wt = wpool.tile([128, 9, 128], f32r, tag="w", name="w",
                                    bufs=4)
                    nc.sync.dma_start(wt[:], WQ[4, :, kc])
                    wq4.append(wt)
                for kc in range(2):
                    nc.scalar.dma_start(x_stg[kc][:],
                                        X[kc].rearrange("p r c -> p (r c)"))
                for r0 in range(0, 64, 16):
                    for kc in range(2):
                        xv = x_pad[kc][:].rearrange(
                            "p (r c) -> p r c", r=66, c=66)
                        xs = x_stg[kc][:].rearrange("p (r c) -> p r c",
                                                    r=64, c=64)
                        nc.vector.tensor_copy(
                            xv[:, r0 + 1:r0 + 17, 1:65],
                            xs[:, r0:r0 + 16, :])

                def conv_group(mc, wts=None, extra=None):
                    if wts is None:
                        wts = []
                        for kc in range(2):
                            wt = wpool.tile([128, 9, 128], f32r, tag="w",
                                            name="w", bufs=4)
                            nc.sync.dma_start(wt[:], WQ[mc, :, kc])
                            wts.append(wt)
                    for rb in range(8):
                        ps_t = cps.tile([128, 512], f32, tag="cps",
                                        name="cpst")
                        psv = ps_t[:].rearrange("p (r c) -> p r c", r=8, c=64)
                        i = 0
                        for kc in range(2):
                            xv = x_pad[kc][:].rearrange(
                                "p (r c) -> p r c", r=66, c=66)
                            for s in range(9):
                                ky, kx = s // 3, s % 3
                                rhs = xv[:, 8 * rb + ky: 8 * rb + ky + 8,
                                         kx: kx + 64]
                                nc.tensor.matmul(
                                    psv, wts[kc][:, s, :], rhs,
                                    start=(i == 0), stop=(i == 17))
                                i += 1
                        bias = bq[:, mc: mc + 1]
                        if mc < 2:
                            dst = q_sb[mc][:, 512 * rb: 512 * (rb + 1)]
                            nc.scalar.add(dst, ps_t[:], bias)
                        elif mc < 4:
                            dst = k_sb[mc - 2][:, 512 * rb: 512 * (rb + 1)]
                            nc.scalar.add(dst, ps_t[:], bias)
                        else:
                            vv = v_pad[mc - 4][:].rearrange(
                                "p (r c) -> p r c", r=66, c=66)
                            dst = vv[:, 8 * rb + 1: 8 * rb + 9, 1:65]
                            nc.scalar.add(dst, psv, bias)
                        if extra is not None:
                            extra(rb)

                # transposed v chunk builder: DVE staging copy + one PE
                # transpose + 2 DVE copies per (ch, cc)
                def make_vt(ch):
                    vtc = vts[ch]
                    for cc in range(2):
                        vv = v_pad[cc][:].rearrange(
                            "p (r c) -> p r c", r=66, c=66)
                        vstg = wpool.tile([128, 128], bf16, tag="vstg",
                                          name="vstg", bufs=4)
                        nc.vector.tensor_copy(
                            vstg[:].rearrange("p (r c) -> p r c", r=2, c=64),
                            vv[:, 2 * ch + 1: 2 * ch + 3, 1:65])
                        tp = tr_ps.tile([128, 128], bf16, tag="tr",
                                        name="trt")
                        nc.tensor.transpose(tp[:], vstg[:], idn[:])
                        nc.vector.tensor_copy(
                            vtc[:, 130 * cc:130 * cc + 64], tp[:, 0:64])
                        nc.vector.tensor_copy(
                            vtc[:, 130 * cc + 66:130 * cc + 130],
                            tp[:, 64:128])

                # pe depthwise conv on DVE, accumulating into att_out
                def pe_conv(cc, g):
                    vvf = v_pad[cc][:].rearrange(
                        "p (r c) -> p r c", r=66, c=66)
                    aof = att_out[cc][:].rearrange(
                        "p (r c) -> p r c", r=64, c=64)
                    r0 = 16 * g
                    dst = aof[:, r0:r0 + 16, :]
                    for s in range(9):
                        ky, kx = s // 3, s % 3
                        sv = vvf[:, r0 + ky: r0 + ky + 16, kx: kx + 64]
                        if s == 0:
                            nc.vector.tensor_scalar_mul(
                                dst, sv, pew[:, cc, 0:1])
                        else:
                            nc.vector.scalar_tensor_tensor(
                                dst, sv, pew[:, cc, s:s + 1], dst,
                                ALU.mult, ALU.add)

                # v first
                conv_group(4, wts=wq4)
                for i in range(32):
                    for cc in range(2):
                        nc.vector.memset(
                            vts[i][:, 130 * cc + 64:130 * cc + 66], 1.0)
                conv_group(5)

                # during q/k conv: transposes spread 1 per rb, pe on GpSimd
                nvt = [0]

                def vt_extra(rb):
                    if nvt[0] < 32:
                        make_vt(nvt[0])
                        nvt[0] += 1

                conv_group(0, extra=vt_extra)
                pe_conv(0, 0)
                pe_conv(0, 1)
                conv_group(1, extra=vt_extra)
                pe_conv(0, 2)
                pe_conv(0, 3)

                # pooling + block-diag a (both dtypes)
                for ccq in range(2):
                    qv = q_sb[ccq][:].rearrange(
                        "p (by dy bx dx) -> p by bx dy dx",
                        by=8, dy=8, bx=8, dx=8)
                    nc.vector.tensor_reduce(a_sum[ccq], qv, AX.XY, ALU.add)
                nc.vector.memset(abd_bf_t[:], 0.0)
                nc.vector.memset(abd_f_t[:].bitcast(f32), 0.0)
                for cc in range(2):
                    for j in range(4):
                        nc.vector.tensor_copy(
                            abd_bf[cc][32 * j:32 * j + 32,
                                       64 * j:64 * j + 64],
                            a_sum[cc][32 * j:32 * j + 32, :])
                        nc.vector.tensor_copy(
                            abd_f_t[32 * j:32 * j + 32,
                                    256 * cc + 64 * j:256 * cc + 64 * j + 64],
                            a_sum[cc][32 * j:32 * j + 32, :])

                # k
                conv_group(2, extra=vt_extra)
                pe_conv(1, 0)
                pe_conv(1, 1)
                conv_group(3, extra=vt_extra)
                pe_conv(1, 2)
                pe_conv(1, 3)
                while nvt[0] < 32:
                    make_vt(nvt[0])
                    nvt[0] += 1

            # ---- stage 1 ----
            # attn_ps[hp] accumulates [128 agents, 66] over 32 chunks:
            # for half 0 cols = [64 ch | Z Z], for half 1 cols = [Z Z | 64 ch]
            with ExitStack() as ph:
                st_ps = ph.enter_context(
                    tc.tile_pool(name="stps", bufs=3, space="PSUM"))
                at_ps = ph.enter_context(
                    tc.tile_pool(name="atps", bufs=4, space="PSUM"))
                etp = ph.enter_context(tc.tile_pool(name="etp", bufs=1))
                attn_ps = [at_ps.tile([128, 66], f32, tag="at", name="at")
                           for _ in range(4)]
                for chp in range(16):   # ch pairs
                    for cc in range(2):
                        sp = st_ps.tile([128, 512], f32, tag="st", name="stt")
                        for u in range(2):
                            ch = 2 * chp + u
                            nc.tensor.matmul(
                                sp[:, 256 * u:256 * (u + 1)],
                                k_sb[cc][:, 128 * ch:128 * (ch + 1)],
                                abd_bf[cc], start=True, stop=True,
                                skip_group_check=True)
                        et = etp.tile([128, 512], bf16, tag="et", name="et",
                                      bufs=4)
                        nc.scalar.activation(et[:], sp[:], AF.Exp, scale=SCALE)
                        for u in range(2):
                            ch = 2 * chp + u
                            for half in range(2):
                                hp = 2 * cc + half
                                rhs = vts[ch][:, 130 * cc + 64 * half:
                                              130 * cc + 64 * half + 66]
                                nc.tensor.matmul(
                                    attn_ps[hp][:],
                                    et[:, 256 * u + 128 * half:
                                       256 * u + 128 * (half + 1)],
                                    rhs, start=(ch == 0), stop=(ch == 31))

                # normalize stage-1 rows by Z1 -> attnZ [128 agents, 64]
                for hp in range(4):
                    half = hp % 2
                    zc = 64 if half == 0 else 0
                    och = 0 if half == 0 else 2
                    r1 = etp.tile([128, 1], f32, tag="r1", name="r1", bufs=4)
                    nc.vector.reciprocal(r1[:], attn_ps[hp][:, zc:zc + 1])
                    nc.vector.memset(attnZ[hp], 0.0)
                    nc.vector.tensor_scalar_mul(
                        attnZ[hp][0:64, 0:32],
                        attn_ps[hp][0:64, och:och + 32], r1[0:64, :])
                    nc.vector.tensor_scalar_mul(
                        attnZ[hp][64:128, 32:64],
                        attn_ps[hp][64:128, och + 32:och + 64],
                        r1[64:128, :])

            # ---- stage 2 + proj ----
            with ExitStack() as ph:
                s2sb = ph.enter_context(tc.tile_pool(name="s2sb", bufs=4))
                osb = ph.enter_context(tc.tile_pool(name="osb", bufs=3))
                s2_ps = ph.enter_context(
                    tc.tile_pool(name="s2ps", bufs=2, space="PSUM"))
                g_ps = ph.enter_context(
                    tc.tile_pool(name="gps", bufs=2, space="PSUM"))
                z_ps = ph.enter_context(
                    tc.tile_pool(name="zps", bufs=2, space="PSUM"))
                pr_ps = ph.enter_context(
                    tc.tile_pool(name="prps", bufs=2, space="PSUM"))

                for nt in range(8):
                    for cc in range(2):
                        gp = g_ps.tile([128, 512], f32, tag="g", name="gt")
                        zp = z_ps.tile([128, 512], f32, tag="z", name="zt")
                        for half in range(2):
                            hp = 2 * cc + half
                            sp = s2_ps.tile([128, 512], f32, tag="s2",
                                            name="s2t")
                            nc.tensor.matmul(
                                sp[:],
                                abd_f_t[:, 256 * cc + 128 * half:
                                        256 * cc + 128 * (half + 1)],
                                q_sb[cc][:, 512 * nt:512 * (nt + 1)],
                                start=True, stop=True)
                            e2 = s2sb.tile([128, 512], bf16, tag="e2",
                                           name="e2")
                            nc.scalar.activation(e2[:], sp[:], AF.Exp,
                                                 scale=SCALE)
                            # g rows 0:64 (half 0) / 64:128 (half 1)
                            nc.tensor.matmul(
                                gp[64 * half:64 * half + 64, :],
                                attnZ[hp], e2[:], start=True, stop=True,
                                skip_group_check=True)
                            # Zb rows: per-head agent sums of e2, already
                            # replicated to each head's 32 d-partitions
                            nc.tensor.matmul(
                                zp[64 * half:64 * half + 64, :],
                                hsel[:], e2[:], start=True, stop=True,
                                skip_group_check=True)
                        rb = s2sb.tile([128, 512], f32, tag="rb", name="rbt")
                        nc.vector.reciprocal_approx_fast(rb[:], zp[:])
                        tsc = s2sb.tile([128, 512], f32r, tag="ts",
                                        name="tsc")
                        nc.vector.tensor_tensor(tsc[:], gp[:], rb[:],
                                                ALU.mult)
                        sl = att_out[cc][:, 512 * nt:512 * (nt + 1)]
                        nc.vector.tensor_tensor(sl, tsc[:].bitcast(f32),
                                                sl.bitcast(f32), ALU.add)
                    for mc in range(2):
                        pp = pr_ps.tile([128, 512], f32, tag="tp", name="prt")
                        for kc in range(2):
                            nc.tensor.matmul(
                                pp[:], pwv[:, kc, 128 * mc:128 * (mc + 1)],
                                att_out[kc][:, 512 * nt:512 * (nt + 1)],
                                start=(kc == 0), stop=(kc == 1))
                        ot = osb.tile([128, 512], f32, tag="ot", name="ott")
                        nc.vector.tensor_scalar_add(ot[:], pp[:],
                                                    pb[:, mc:mc + 1])
                        nc.sync.dma_start(
                            OUT[mc, :, 512 * nt:512 * (nt + 1)], ot[:])

    nc.compile()
    return nc


def _prep_consts(qkv_w, qkv_s, qkv_b, pe_w, pe_s, pe_b, proj_w, proj_s,
                 proj_b):
    import ml_dtypes
    f = np.float32
    bf = ml_dtypes.bfloat16
    w = np.asarray(qkv_w, f).copy()          # [768, 256, 3, 3]
    dif = (w[:, :, 0, 1] + w[:, :, 1, 0] + w[:, :, 1, 1] + w[:, :, 1, 2]
           + w[:, :, 2, 1])
    w[:, :, 1, 1] -= THETA * dif
    w *= np.asarray(qkv_s, f)[:, None, None, None]
    # WQ[mc, p, kc, s, o'] = w[128*mc+o', 128*kc+p, s//3, s%3]
    wq = w.reshape(6, 128, 2, 128, 9)        # [mc, o', kc, p, s]
    wq = np.ascontiguousarray(wq.transpose(0, 3, 2, 4, 1))  # [6,128,2,9,128]

    bq = np.ascontiguousarray(np.asarray(qkv_b, f).reshape(6, 128).T)

    pe_wf = np.asarray(pe_w, f)[:, 0] * np.asarray(pe_s, f)[:, None, None]
    pew = np.zeros((128, 2, 9), f)
    for kc in range(2):
        for s in range(9):
            pew[:, kc, s] = pe_wf[128 * kc:128 * (kc + 1), s // 3, s % 3]

    pwm = np.asarray(proj_w, f)[:, :, 0, 0] * np.asarray(proj_s, f)[:, None]
    pw = np.ascontiguousarray(
        pwm.T.reshape(2, 128, 256).transpose(1, 0, 2).reshape(128, 512))
    pbv = np.asarray(proj_b, f) + pwm @ np.asarray(pe_b, f)
    pb = np.ascontiguousarray(pbv.reshape(2, 128).T)

    idn = np.eye(128, dtype=bf)
    return dict(wq=wq, bq=bq, pew=pew, pw=pw, pb=pb, idn=idn)


def kernel(x, qkv_w, qkv_s, qkv_b, pe_w, pe_s, pe_b, proj_w, proj_s, proj_b):
    import ml_dtypes
    from concourse.bass_utils import run_bass_kernel_spmd

    if "nc" not in _cache:
        _cache["nc"] = _build()
    nc = _cache["nc"]

    consts = _prep_consts(qkv_w, qkv_s, qkv_b, pe_w, pe_s, pe_b, proj_w,
                          proj_s, proj_b)
    x = np.asarray(x, np.float32).astype(ml_dtypes.bfloat16)
    in_maps = []
    for b in range(B):
        m = dict(consts)
        m["x"] = np.ascontiguousarray(x[b].reshape(2, 128, H, W))
        in_maps.append(m)

    res = run_bass_kernel_spmd(nc, in_maps, list(range(N_CORES)), trace=False)
    out = np.empty((B, C, H, W), np.float32)
    for b in range(B):
        out[b] = res.results[b]["out"].reshape(C, H, W)
    return out


# revision 24
# speedup vs baseline: 1.1790x; 1.1735x over previous
"""Trainium2 Bass kernel for agent attention (sparse_attention problem).

Per-core work (data-parallel over batch B=8 across 8 NeuronCores):
  x[b] [256, 64, 64] -> qkv 3x3 conv (dif-conv + BN folded into weights)
  -> agent attention (8 heads, d=32, 64 agent tokens)
  -> depthwise 3x3 pe conv on v -> 1x1 proj.

v3: f32r conv matmuls (self-loading weights, no exposed LDWEIGHTS),
bf16 input staging for fast DMA, GpSimd depthwise pe conv, paired
stage-1 exps, restructured stage-2 (g = attnZ^T @ e2 full-width
matmuls, matmul-broadcast softmax denominator, fast reciprocal).
"""
import numpy as np

NUM_HEADS = 8
AGENT_NUM = 64
THETA = 0.7
C = 256
H = W = 64
HW = H * W
D = C // NUM_HEADS          # 32
N_AG = AGENT_NUM            # 64
PS = 8                      # pool size
N_CORES = 8
B = 8

_cache = {}


def _build():
    import concourse.bass as bass
    import concourse.tile as tile
    from concourse import bacc, mybir

    f32 = mybir.dt.float32
    f32r = mybir.dt.float32r
    bf16 = mybir.dt.bfloat16
    AF = mybir.ActivationFunctionType
    ALU = mybir.AluOpType
    AX = mybir.AxisListType

    nc = bacc.Bacc("TRN2", target_bir_lowering=False, debug=False,
                   enable_asserts=True, num_devices=N_CORES)

    X = nc.dram_tensor("x", [2, 128, H, W], bf16, kind="ExternalInput").ap()
    WQ = nc.dram_tensor("wq", [6, 128, 2, 9, 128], f32r,
                        kind="ExternalInput").ap()
    BQ = nc.dram_tensor("bq", [128, 6], f32, kind="ExternalInput").ap()
    PEW = nc.dram_tensor("pew", [128, 2, 9], f32, kind="ExternalInput").ap()
    PW = nc.dram_tensor("pw", [128, 2 * 256], f32r, kind="ExternalInput").ap()
    PB = nc.dram_tensor("pb", [128, 2], f32, kind="ExternalInput").ap()
    IDN = nc.dram_tensor("idn", [128, 128], bf16, kind="ExternalInput").ap()
    OUT = nc.dram_tensor("out", [2, 128, HW], f32, kind="ExternalOutput").ap()

    # softmax exp scale: d^-0.5, with the 1/64 agent-pool mean folded in
    SCALE = (D ** -0.5) / (PS * PS)

    with tile.TileContext(nc) as tc:
        from contextlib import ExitStack
        with ExitStack() as top:
            pers = top.enter_context(tc.tile_pool(name="pers", bufs=1))
            q_sb = [pers.tile([128, HW], f32r, tag=f"q{i}", name=f"q{i}")
                    for i in range(2)]
            k_sb = [pers.tile([128, HW], bf16, tag=f"k{i}", name=f"k{i}")
                    for i in range(2)]
            v_pad = [pers.tile([128, 66 * 66], bf16, tag=f"vp{i}",
                               name=f"vp{i}") for i in range(2)]
            att_out = [pers.tile([128, HW], f32r, tag=f"ao{i}", name=f"ao{i}")
                       for i in range(2)]
            bq = pers.tile([128, 6], f32, tag="bq", name="bq")
            idn = pers.tile([128, 128], bf16, tag="idn", name="idn")
            pew = pers.tile([128, 2, 9], f32, tag="pew", name="pew")
            asum_t = pers.tile([128, 128], f32, tag="asum", name="asum")
            a_sum = [asum_t[:, 64 * i:64 * (i + 1)] for i in range(2)]
            # a_bd4 needed in bf16 (stage-1 rhs) and f32r (stage-2 lhsT)
            abd_bf_t = pers.tile([128, 512], bf16, tag="abdb", name="abdb")
            abd_bf = [abd_bf_t[:, 256 * i:256 * (i + 1)] for i in range(2)]
            abd_f_t = pers.tile([128, 512], f32r, tag="abdf", name="abdf")
            az_t = pers.tile([128, 4 * 64], bf16, tag="az", name="az")
            attnZ = [az_t[:, 64 * i:64 * (i + 1)] for i in range(4)]
            pw = pers.tile([128, 2 * 256], f32r, tag="pw", name="pwt")
            pb = pers.tile([128, 2], f32, tag="pb", name="pbt")
            hsel = pers.tile([128, 64], bf16, tag="hsel", name="hsel")
            nc.sync.dma_start(bq[:], BQ[:])
            nc.sync.dma_start(idn[:], IDN[:])
            nc.sync.dma_start(pew[:], PEW[:])
            nc.sync.dma_start(pw[:], PW[:])
            nc.sync.dma_start(pb[:], PB[:])
            pwv = pw[:].rearrange("p (a b) -> p a b", a=2, b=256)

            # hsel: block-ones selector so hsel^T @ e2 replicates each
            # head's agent-sum across that head's 32 d-partitions
            nc.vector.memset(hsel[:], 0.0)
            nc.vector.memset(hsel[0:64, 0:32], 1.0)
            nc.vector.memset(hsel[64:128, 32:64], 1.0)

            for cc in range(2):
                vv = v_pad[cc][:].rearrange("p (r c) -> p r c", r=66, c=66)
                nc.vector.memset(vv[:, 0:1, :], 0.0)
                nc.vector.memset(vv[:, 65:66, :], 0.0)
                nc.vector.memset(vv[:, :, 0:1], 0.0)
                nc.vector.memset(vv[:, :, 65:66], 0.0)

            # persistent v^T chunk tiles (layout per cc-section of 130:
            # [64 ch of hp-even | 2 ones | 64 ch of hp-odd])
            s1sb = top.enter_context(tc.tile_pool(name="s1sb", bufs=1))
            vts = [s1sb.tile([128, 260], bf16, tag=f"vt{i}", name=f"vt{i}")
                   for i in range(32)]

            with ExitStack() as ph:
                tr_ps = ph.enter_context(
                    tc.tile_pool(name="trps", bufs=2, space="PSUM"))
                cpool = ph.enter_context(tc.tile_pool(name="conv", bufs=1))
                wpool = ph.enter_context(tc.tile_pool(name="wq", bufs=1))
                cps = ph.enter_context(
                    tc.tile_pool(name="cps", bufs=4, space="PSUM"))

                x_pad = [cpool.tile([128, 66 * 66], f32r, tag=f"xp{i}",
                                    name=f"xp{i}") for i in range(2)]
                x_stg = [cpool.tile([128, HW], bf16, tag=f"xs{i}",
                                    name=f"xs{i}") for i in range(2)]
                for kc in range(2):
                    xv = x_pad[kc][:].bitcast(f32).rearrange(
                        "p (r c) -> p r c", r=66, c=66)
                    nc.vector.memset(xv[:, 0:1, :], 0.0)
                    nc.vector.memset(xv[:, 65:66, :], 0.0)
                    nc.vector.memset(xv[:, :, 0:1], 0.0)
                    nc.vector.memset(xv[:, :, 65:66], 0.0)

                # first conv group's weights ahead of the bulk x transfer;
                # x lands as one fully-contiguous transfer per group and
                # DVE pads/casts it to f32
                wq4 = []
                for kc in range(2):
                    wt = wpool.tile([128, 9, 128], f32r, tag="w", name="w",
                                    bufs=4)
                    nc.sync.dma_start(wt[:], WQ[4, :, kc])
                    wq4.append(wt)
                for kc in range(2):
                    nc.sync.dma_start(x_stg[kc][:],
                                      X[kc].rearrange("p r c -> p (r c)"))
                for r0 in range(0, 64, 16):
                    for kc in range(2):
                        xv = x_pad[kc][:].rearrange(
                            "p (r c) -> p r c", r=66, c=66)
                        xs = x_stg[kc][:].rearrange("p (r c) -> p r c",
                                                    r=64, c=64)
                        nc.vector.tensor_copy(
                            xv[:, r0 + 1:r0 + 17, 1:65],
                            xs[:, r0:r0 + 16, :])

                def conv_group(mc, wts=None, extra=None):
                    if wts is None:
                        wts = []
                        for kc in range(2):
                            wt = wpool.tile([128, 9, 128], f32r, tag="w",
                                            name="w", bufs=4)
                            nc.sync.dma_start(wt[:], WQ[mc, :, kc])
                            wts.append(wt)
                    for rb in range(8):
                        ps_t = cps.tile([128, 512], f32, tag="cps",
                                        name="cpst")
                        psv = ps_t[:].rearrange("p (r c) -> p r c", r=8, c=64)
                        i = 0
                        for kc in range(2):
                            xv = x_pad[kc][:].rearrange(
                                "p (r c) -> p r c", r=66, c=66)
                            for s in range(9):
                                ky, kx = s // 3, s % 3
                                rhs = xv[:, 8 * rb + ky: 8 * rb + ky + 8,
                                         kx: kx + 64]
                                nc.tensor.matmul(
                                    psv, wts[kc][:, s, :], rhs,
                                    start=(i == 0), stop=(i == 17))
                                i += 1
                        bias = bq[:, mc: mc + 1]
                        if mc < 2:
                            dst = q_sb[mc][:, 512 * rb: 512 * (rb + 1)]
                            nc.scalar.add(dst, ps_t[:], bias)
                        elif mc < 4:
                            dst = k_sb[mc - 2][:, 512 * rb: 512 * (rb + 1)]
                            nc.scalar.add(dst, ps_t[:], bias)
                        else:
                            vv = v_pad[mc - 4][:].rearrange(
                                "p (r c) -> p r c", r=66, c=66)
                            dst = vv[:, 8 * rb + 1: 8 * rb + 9, 1:65]
                            nc.scalar.add(dst, psv, bias)
                        if extra is not None:
                            extra(rb)

                # transposed v chunk builder: DVE staging copy + one PE
                # transpose + 2 DVE copies per (ch, cc)
                def make_vt(ch):
                    vtc = vts[ch]
                    for cc in range(2):
                        vv = v_pad[cc][:].rearrange(
                            "p (r c) -> p r c", r=66, c=66)
                        vstg = wpool.tile([128, 128], bf16, tag="vstg",
                                          name="vstg", bufs=4)
                        nc.vector.tensor_copy(
                            vstg[:].rearrange("p (r c) -> p r c", r=2, c=64),
                            vv[:, 2 * ch + 1: 2 * ch + 3, 1:65])
                        tp = tr_ps.tile([128, 128], bf16, tag="tr",
                                        name="trt")
                        nc.tensor.transpose(tp[:], vstg[:], idn[:])
                        nc.vector.tensor_copy(
                            vtc[:, 130 * cc:130 * cc + 64], tp[:, 0:64])
                        nc.vector.tensor_copy(
                            vtc[:, 130 * cc + 66:130 * cc + 130],
                            tp[:, 64:128])

                # pe depthwise conv on DVE, accumulating into att_out
                def pe_conv(cc, g):
                    vvf = v_pad[cc][:].rearrange(
                        "p (r c) -> p r c", r=66, c=66)
                    aof = att_out[cc][:].rearrange(
                        "p (r c) -> p r c", r=64, c=64)
                    r0 = 16 * g
                    dst = aof[:, r0:r0 + 16, :]
                    for s in range(9):
                        ky, kx = s // 3, s % 3
                        sv = vvf[:, r0 + ky: r0 + ky + 16, kx: kx + 64]
                        if s == 0:
                            nc.vector.tensor_scalar_mul(
                                dst, sv, pew[:, cc, 0:1])
                        else:
                            nc.vector.scalar_tensor_tensor(
                                dst, sv, pew[:, cc, s:s + 1], dst,
                                ALU.mult, ALU.add)

                # v first
                conv_group(4, wts=wq4)
                for i in range(32):
                    for cc in range(2):
                        nc.vector.memset(
                            vts[i][:, 130 * cc + 64:130 * cc + 66], 1.0)
                conv_group(5)

                # during q/k conv: transposes spread 1 per rb, pe on GpSimd
                nvt = [0]

                def vt_extra(rb):
                    if nvt[0] < 32:
                        make_vt(nvt[0])
                        nvt[0] += 1

                conv_group(0, extra=vt_extra)
                pe_conv(0, 0)
                pe_conv(0, 1)
                conv_group(1, extra=vt_extra)
                pe_conv(0, 2)
                pe_conv(0, 3)

                # pooling + block-diag a (both dtypes)
                for ccq in range(2):
                    qv = q_sb[ccq][:].rearrange(
                        "p (by dy bx dx) -> p by bx dy dx",
                        by=8, dy=8, bx=8, dx=8)
                    nc.vector.tensor_reduce(a_sum[ccq], qv, AX.XY, ALU.add)
                nc.vector.memset(abd_bf_t[:], 0.0)
                nc.vector.memset(abd_f_t[:].bitcast(f32), 0.0)
                for cc in range(2):
                    for j in range(4):
                        nc.vector.tensor_copy(
                            abd_bf[cc][32 * j:32 * j + 32,
                                       64 * j:64 * j + 64],
                            a_sum[cc][32 * j:32 * j + 32, :])
                        nc.vector.tensor_copy(
                            abd_f_t[32 * j:32 * j + 32,
                                    256 * cc + 64 * j:256 * cc + 64 * j + 64],
                            a_sum[cc][32 * j:32 * j + 32, :])

                # k
                conv_group(2, extra=vt_extra)
                pe_conv(1, 0)
                pe_conv(1, 1)
                conv_group(3, extra=vt_extra)
                pe_conv(1, 2)
                pe_conv(1, 3)
                while nvt[0] < 32:
                    make_vt(nvt[0])
                    nvt[0] += 1

            # ---- stage 1 ----
            # attn_ps[hp] accumulates [128 agents, 66] over 32 chunks:
            # for half 0 cols = [64 ch | Z Z], for half 1 cols = [Z Z | 64 ch]
            with ExitStack() as ph:
                st_ps = ph.enter_context(
                    tc.tile_pool(name="stps", bufs=3, space="PSUM"))
                at_ps = ph.enter_context(
                    tc.tile_pool(name="atps", bufs=4, space="PSUM"))
                etp = ph.enter_context(tc.tile_pool(name="etp", bufs=1))
                attn_ps = [at_ps.tile([128, 66], f32, tag="at", name="at")
                           for _ in range(4)]
                for chp in range(16):   # ch pairs
                    for cc in range(2):
                        sp = st_ps.tile([128, 512], f32, tag="st", name="stt")
                        for u in range(2):
                            ch = 2 * chp + u
                            nc.tensor.matmul(
                                sp[:, 256 * u:256 * (u + 1)],
                                k_sb[cc][:, 128 * ch:128 * (ch + 1)],
                                abd_bf[cc], start=True, stop=True,
                                skip_group_check=True)
                        et = etp.tile([128, 512], bf16, tag="et", name="et",
                                      bufs=4)
                        nc.scalar.activation(et[:], sp[:], AF.Exp, scale=SCALE)
                        for u in range(2):
                            ch = 2 * chp + u
                            for half in range(2):
                                hp = 2 * cc + half
                                rhs = vts[ch][:, 130 * cc + 64 * half:
                                              130 * cc + 64 * half + 66]
                                nc.tensor.matmul(
                                    attn_ps[hp][:],
                                    et[:, 256 * u + 128 * half:
                                       256 * u + 128 * (half + 1)],
                                    rhs, start=(ch == 0), stop=(ch == 31))

                # normalize stage-1 rows by Z1 -> attnZ [128 agents, 64]
                for hp in range(4):
                    half = hp % 2
                    zc = 64 if half == 0 else 0
                    och = 0 if half == 0 else 2
                    r1 = etp.tile([128, 1], f32, tag="r1", name="r1", bufs=4)
                    nc.vector.reciprocal(r1[:], attn_ps[hp][:, zc:zc + 1])
                    nc.vector.memset(attnZ[hp], 0.0)
                    nc.vector.tensor_scalar_mul(
                        attnZ[hp][0:64, 0:32],
                        attn_ps[hp][0:64, och:och + 32], r1[0:64, :])
                    nc.vector.tensor_scalar_mul(
                        attnZ[hp][64:128, 32:64],
                        attn_ps[hp][64:128, och + 32:och + 64],
                        r1[64:128, :])

            # ---- stage 2 + proj ----
            with ExitStack() as ph:
                s2sb = ph.enter_context(tc.tile_pool(name="s2sb", bufs=4))
                osb = ph.enter_context(tc.tile_pool(name="osb", bufs=3))
                s2_ps = ph.enter_context(
                    tc.tile_pool(name="s2ps", bufs=2, space="PSUM"))
                g_ps = ph.enter_context(
                    tc.tile_pool(name="gps", bufs=2, space="PSUM"))
                z_ps = ph.enter_context(
                    tc.tile_pool(name="zps", bufs=2, space="PSUM"))
                pr_ps = ph.enter_context(
                    tc.tile_pool(name="prps", bufs=2, space="PSUM"))

                for nt in range(8):
                    for cc in range(2):
                        gp = g_ps.tile([128, 512], f32, tag="g", name="gt")
                        zp = z_ps.tile([128, 512], f32, tag="z", name="zt")
                        for half in range(2):
                            hp = 2 * cc + half
                            sp = s2_ps.tile([128, 512], f32, tag="s2",
                                            name="s2t")
                            nc.tensor.matmul(
                                sp[:],
                                abd_f_t[:, 256 * cc + 128 * half:
                                        256 * cc + 128 * (half + 1)],
                                q_sb[cc][:, 512 * nt:512 * (nt + 1)],
                                start=True, stop=True)
                            e2 = s2sb.tile([128, 512], bf16, tag="e2",
                                           name="e2")
                            nc.scalar.activation(e2[:], sp[:], AF.Exp,
                                                 scale=SCALE)
                            # g rows 0:64 (half 0) / 64:128 (half 1)
                            nc.tensor.matmul(
                                gp[64 * half:64 * half + 64, :],
                                attnZ[hp], e2[:], start=True, stop=True,
                                skip_group_check=True)
                            # Zb rows: per-head agent sums of e2, already
                            # replicated to each head's 32 d-partitions
                            nc.tensor.matmul(
                                zp[64 * half:64 * half + 64, :],
                                hsel[:], e2[:], start=True, stop=True,
                                skip_group_check=True)
                        rb = s2sb.tile([128, 512], f32, tag="rb", name="rbt")
                        nc.vector.reciprocal_approx_fast(rb[:], zp[:])
                        tsc = s2sb.tile([128, 512], f32r, tag="ts",
                                        name="tsc")
                        nc.vector.tensor_tensor(tsc[:], gp[:], rb[:],
                                                ALU.mult)
                        sl = att_out[cc][:, 512 * nt:512 * (nt + 1)]
                        nc.vector.tensor_tensor(sl, tsc[:].bitcast(f32),
                                                sl.bitcast(f32), ALU.add)
                    for mc in range(2):
                        pp = pr_ps.tile([128, 512], f32, tag="tp", name="prt")
                        for kc in range(2):
                            nc.tensor.matmul(
                                pp[:], pwv[:, kc, 128 * mc:128 * (mc + 1)],
                                att_out[kc][:, 512 * nt:512 * (nt + 1)],
                                start=(kc == 0), stop=(kc == 1))
                        ot = osb.tile([128, 512], f32, tag="ot", name="ott")
                        nc.vector.tensor_scalar_add(ot[:], pp[:],
                                                    pb[:, mc:mc + 1])
                        nc.sync.dma_start(
                            OUT[mc, :, 512 * nt:512 * (nt + 1)], ot[:])

    nc.compile()
    return nc


def _prep_consts(qkv_w, qkv_s, qkv_b, pe_w, pe_s, pe_b, proj_w, proj_s,
                 proj_b):
    import ml_dtypes
    f = np.float32
    bf = ml_dtypes.bfloat16
    w = np.asarray(qkv_w, f).copy()          # [768, 256, 3, 3]
    dif = (w[:, :, 0, 1] + w[:, :, 1, 0] + w[:, :, 1, 1] + w[:, :, 1, 2]
           + w[:, :, 2, 1])
    w[:, :, 1, 1] -= THETA * dif
    w *= np.asarray(qkv_s, f)[:, None, None, None]
    # WQ[mc, p, kc, s, o'] = w[128*mc+o', 128*kc+p, s//3, s%3]
    wq = w.reshape(6, 128, 2, 128, 9)        # [mc, o', kc, p, s]
    wq = np.ascontiguousarray(wq.transpose(0, 3, 2, 4, 1))  # [6,128,2,9,128]

    bq = np.ascontiguousarray(np.asarray(qkv_b, f).reshape(6, 128).T)

    pe_wf = np.asarray(pe_w, f)[:, 0] * np.asarray(pe_s, f)[:, None, None]
    pew = np.zeros((128, 2, 9), f)
    for kc in range(2):
        for s in range(9):
            pew[:, kc, s] = pe_wf[128 * kc:128 * (kc + 1), s // 3, s % 3]

    pwm = np.asarray(proj_w, f)[:, :, 0, 0] * np.asarray(proj_s, f)[:, None]
    pw = np.ascontiguousarray(
        pwm.T.reshape(2, 128, 256).transpose(1, 0, 2).reshape(128, 512))
    pbv = np.asarray(proj_b, f) + pwm @ np.asarray(pe_b, f)
    pb = np.ascontiguousarray(pbv.reshape(2, 128).T)

    idn = np.eye(128, dtype=bf)
    return dict(wq=wq, bq=bq, pew=pew, pw=pw, pb=pb, idn=idn)


def kernel(x, qkv_w, qkv_s, qkv_b, pe_w, pe_s, pe_b, proj_w, proj_s, proj_b):
    import ml_dtypes
    from concourse.bass_utils import run_bass_kernel_spmd

    if "nc" not in _cache:
        _cache["nc"] = _build()
    nc = _cache["nc"]

    consts = _prep_consts(qkv_w, qkv_s, qkv_b, pe_w, pe_s, pe_b, proj_w,
                          proj_s, proj_b)
    x = np.asarray(x, np.float32).astype(ml_dtypes.bfloat16)
    in_maps = []
    for b in range(B):
        m = dict(consts)
        m["x"] = np.ascontiguousarray(x[b].reshape(2, 128, H, W))
        in_maps.append(m)

    res = run_bass_kernel_spmd(nc, in_maps, list(range(N_CORES)), trace=False)
    out = np.empty((B, C, H, W), np.float32)
    for b in range(B):
        out[b] = res.results[b]["out"].reshape(C, H, W)
    return out


# revision 25
# speedup vs baseline: 1.2020x; 1.0195x over previous
"""Trainium2 Bass kernel for agent attention (sparse_attention problem).

Per-core work (data-parallel over batch B=8 across 8 NeuronCores):
  x[b] [256, 64, 64] -> qkv 3x3 conv (dif-conv + BN folded into weights)
  -> agent attention (8 heads, d=32, 64 agent tokens)
  -> depthwise 3x3 pe conv on v -> 1x1 proj.

v3: f32r conv matmuls (self-loading weights, no exposed LDWEIGHTS),
bf16 input staging for fast DMA, GpSimd depthwise pe conv, paired
stage-1 exps, restructured stage-2 (g = attnZ^T @ e2 full-width
matmuls, matmul-broadcast softmax denominator, fast reciprocal).
"""
import numpy as np

NUM_HEADS = 8
AGENT_NUM = 64
THETA = 0.7
C = 256
H = W = 64
HW = H * W
D = C // NUM_HEADS          # 32
N_AG = AGENT_NUM            # 64
PS = 8                      # pool size
N_CORES = 8
B = 8

_cache = {}


def _build():
    import concourse.bass as bass
    import concourse.tile as tile
    from concourse import bacc, mybir

    f32 = mybir.dt.float32
    f32r = mybir.dt.float32r
    bf16 = mybir.dt.bfloat16
    AF = mybir.ActivationFunctionType
    ALU = mybir.AluOpType
    AX = mybir.AxisListType

    nc = bacc.Bacc("TRN2", target_bir_lowering=False, debug=False,
                   enable_asserts=True, num_devices=N_CORES)

    X = nc.dram_tensor("x", [2, 128, H, W], bf16, kind="ExternalInput").ap()
    WQ = nc.dram_tensor("wq", [6, 128, 2, 9, 128], f32r,
                        kind="ExternalInput").ap()
    BQ = nc.dram_tensor("bq", [128, 6], f32, kind="ExternalInput").ap()
    PEW = nc.dram_tensor("pew", [128, 2, 9], f32, kind="ExternalInput").ap()
    PW = nc.dram_tensor("pw", [128, 2 * 256], f32r, kind="ExternalInput").ap()
    PB = nc.dram_tensor("pb", [128, 2], f32, kind="ExternalInput").ap()
    IDN = nc.dram_tensor("idn", [128, 128], bf16, kind="ExternalInput").ap()
    OUT = nc.dram_tensor("out", [2, 128, HW], f32, kind="ExternalOutput").ap()

    # softmax exp scale: d^-0.5, with the 1/64 agent-pool mean folded in
    SCALE = (D ** -0.5) / (PS * PS)

    with tile.TileContext(nc) as tc:
        from contextlib import ExitStack
        with ExitStack() as top:
            pers = top.enter_context(tc.tile_pool(name="pers", bufs=1))
            q_sb = [pers.tile([128, HW], f32r, tag=f"q{i}", name=f"q{i}")
                    for i in range(2)]
            k_sb = [pers.tile([128, HW], bf16, tag=f"k{i}", name=f"k{i}")
                    for i in range(2)]
            v_pad = [pers.tile([128, 66 * 66], bf16, tag=f"vp{i}",
                               name=f"vp{i}") for i in range(2)]
            att_out = [pers.tile([128, HW], f32r, tag=f"ao{i}", name=f"ao{i}")
                       for i in range(2)]
            bq = pers.tile([128, 6], f32, tag="bq", name="bq")
            idn = pers.tile([128, 128], bf16, tag="idn", name="idn")
            pew = pers.tile([128, 2, 9], f32, tag="pew", name="pew")
            asum_t = pers.tile([128, 128], f32, tag="asum", name="asum")
            a_sum = [asum_t[:, 64 * i:64 * (i + 1)] for i in range(2)]
            # a_bd4 needed in bf16 (stage-1 rhs) and f32r (stage-2 lhsT)
            abd_bf_t = pers.tile([128, 512], bf16, tag="abdb", name="abdb")
            abd_bf = [abd_bf_t[:, 256 * i:256 * (i + 1)] for i in range(2)]
            abd_f_t = pers.tile([128, 512], f32r, tag="abdf", name="abdf")
            az_t = pers.tile([128, 4 * 64], bf16, tag="az", name="az")
            attnZ = [az_t[:, 64 * i:64 * (i + 1)] for i in range(4)]
            pw = pers.tile([128, 2 * 256], f32r, tag="pw", name="pwt")
            pb = pers.tile([128, 2], f32, tag="pb", name="pbt")
            hsel = pers.tile([128, 64], bf16, tag="hsel", name="hsel")
            pwv = pw[:].rearrange("p (a b) -> p a b", a=2, b=256)

            # hsel: block-ones selector so hsel^T @ e2 replicates each
            # head's agent-sum across that head's 32 d-partitions
            nc.vector.memset(hsel[:], 0.0)
            nc.vector.memset(hsel[0:64, 0:32], 1.0)
            nc.vector.memset(hsel[64:128, 32:64], 1.0)

            for cc in range(2):
                vv = v_pad[cc][:].rearrange("p (r c) -> p r c", r=66, c=66)
                nc.vector.memset(vv[:, 0:1, :], 0.0)
                nc.vector.memset(vv[:, 65:66, :], 0.0)
                nc.vector.memset(vv[:, :, 0:1], 0.0)
                nc.vector.memset(vv[:, :, 65:66], 0.0)

            # persistent v^T chunk tiles (layout per cc-section of 130:
            # [64 ch of hp-even | 2 ones | 64 ch of hp-odd])
            s1sb = top.enter_context(tc.tile_pool(name="s1sb", bufs=1))
            vts = [s1sb.tile([128, 260], bf16, tag=f"vt{i}", name=f"vt{i}")
                   for i in range(32)]

            with ExitStack() as ph:
                tr_ps = ph.enter_context(
                    tc.tile_pool(name="trps", bufs=2, space="PSUM"))
                cpool = ph.enter_context(tc.tile_pool(name="conv", bufs=1))
                wpool = ph.enter_context(tc.tile_pool(name="wq", bufs=1))
                cps = ph.enter_context(
                    tc.tile_pool(name="cps", bufs=4, space="PSUM"))

                x_pad = [cpool.tile([128, 66 * 66], f32r, tag=f"xp{i}",
                                    name=f"xp{i}") for i in range(2)]
                x_stg = [cpool.tile([128, HW], bf16, tag=f"xs{i}",
                                    name=f"xs{i}") for i in range(2)]
                for kc in range(2):
                    xv = x_pad[kc][:].bitcast(f32).rearrange(
                        "p (r c) -> p r c", r=66, c=66)
                    nc.vector.memset(xv[:, 0:1, :], 0.0)
                    nc.vector.memset(xv[:, 65:66, :], 0.0)
                    nc.vector.memset(xv[:, :, 0:1], 0.0)
                    nc.vector.memset(xv[:, :, 65:66], 0.0)

                # first conv group's weights ahead of the bulk x transfer;
                # x lands as one fully-contiguous transfer per group and
                # DVE pads/casts it to f32
                wq4 = []
                for kc in range(2):
                    wt = wpool.tile([128, 9, 128], f32r, tag="w", name="w",
                                    bufs=4)
                    nc.sync.dma_start(wt[:], WQ[4, :, kc])
                    wq4.append(wt)
                for kc in range(2):
                    xsv = x_stg[kc][:].rearrange("p (r c) -> p r c",
                                                 r=64, c=64)
                    nc.sync.dma_start(xsv[:, 0:32, :], X[kc, :, 0:32, :])
                for kc in range(2):
                    xsv = x_stg[kc][:].rearrange("p (r c) -> p r c",
                                                 r=64, c=64)
                    nc.sync.dma_start(xsv[:, 32:64, :], X[kc, :, 32:64, :])
                nc.sync.dma_start(bq[:], BQ[:])
                nc.sync.dma_start(idn[:], IDN[:])
                nc.sync.dma_start(pew[:], PEW[:])
                nc.sync.dma_start(pw[:], PW[:])
                nc.sync.dma_start(pb[:], PB[:])
                for r0 in range(0, 64, 16):
                    for kc in range(2):
                        xv = x_pad[kc][:].rearrange(
                            "p (r c) -> p r c", r=66, c=66)
                        xs = x_stg[kc][:].rearrange("p (r c) -> p r c",
                                                    r=64, c=64)
                        nc.vector.tensor_copy(
                            xv[:, r0 + 1:r0 + 17, 1:65],
                            xs[:, r0:r0 + 16, :])

                def conv_group(mc, wts=None, extra=None):
                    if wts is None:
                        wts = []
                        for kc in range(2):
                            wt = wpool.tile([128, 9, 128], f32r, tag="w",
                                            name="w", bufs=4)
                            nc.sync.dma_start(wt[:], WQ[mc, :, kc])
                            wts.append(wt)
                    for rb in range(8):
                        ps_t = cps.tile([128, 512], f32, tag="cps",
                                        name="cpst")
                        psv = ps_t[:].rearrange("p (r c) -> p r c", r=8, c=64)
                        i = 0
                        for kc in range(2):
                            xv = x_pad[kc][:].rearrange(
                                "p (r c) -> p r c", r=66, c=66)
                            for s in range(9):
                                ky, kx = s // 3, s % 3
                                rhs = xv[:, 8 * rb + ky: 8 * rb + ky + 8,
                                         kx: kx + 64]
                                nc.tensor.matmul(
                                    psv, wts[kc][:, s, :], rhs,
                                    start=(i == 0), stop=(i == 17))
                                i += 1
                        bias = bq[:, mc: mc + 1]
                        if mc < 2:
                            dst = q_sb[mc][:, 512 * rb: 512 * (rb + 1)]
                            nc.scalar.add(dst, ps_t[:], bias)
                        elif mc < 4:
                            dst = k_sb[mc - 2][:, 512 * rb: 512 * (rb + 1)]
                            nc.scalar.add(dst, ps_t[:], bias)
                        else:
                            vv = v_pad[mc - 4][:].rearrange(
                                "p (r c) -> p r c", r=66, c=66)
                            dst = vv[:, 8 * rb + 1: 8 * rb + 9, 1:65]
                            nc.scalar.add(dst, psv, bias)
                        if extra is not None:
                            extra(rb)

                # transposed v chunk builder: DVE staging copy + one PE
                # transpose + 2 DVE copies per (ch, cc)
                def make_vt(ch):
                    vtc = vts[ch]
                    for cc in range(2):
                        vv = v_pad[cc][:].rearrange(
                            "p (r c) -> p r c", r=66, c=66)
                        vstg = wpool.tile([128, 128], bf16, tag="vstg",
                                          name="vstg", bufs=4)
                        nc.vector.tensor_copy(
                            vstg[:].rearrange("p (r c) -> p r c", r=2, c=64),
                            vv[:, 2 * ch + 1: 2 * ch + 3, 1:65])
                        tp = tr_ps.tile([128, 128], bf16, tag="tr",
                                        name="trt")
                        nc.tensor.transpose(tp[:], vstg[:], idn[:])
                        nc.vector.tensor_copy(
                            vtc[:, 130 * cc:130 * cc + 64], tp[:, 0:64])
                        nc.vector.tensor_copy(
                            vtc[:, 130 * cc + 66:130 * cc + 130],
                            tp[:, 64:128])

                # pe depthwise conv on DVE, accumulating into att_out
                def pe_conv(cc, g):
                    vvf = v_pad[cc][:].rearrange(
                        "p (r c) -> p r c", r=66, c=66)
                    aof = att_out[cc][:].rearrange(
                        "p (r c) -> p r c", r=64, c=64)
                    r0 = 16 * g
                    dst = aof[:, r0:r0 + 16, :]
                    for s in range(9):
                        ky, kx = s // 3, s % 3
                        sv = vvf[:, r0 + ky: r0 + ky + 16, kx: kx + 64]
                        if s == 0:
                            nc.vector.tensor_scalar_mul(
                                dst, sv, pew[:, cc, 0:1])
                        else:
                            nc.vector.scalar_tensor_tensor(
                                dst, sv, pew[:, cc, s:s + 1], dst,
                                ALU.mult, ALU.add)

                # v first
                conv_group(4, wts=wq4)
                for i in range(32):
                    for cc in range(2):
                        nc.vector.memset(
                            vts[i][:, 130 * cc + 64:130 * cc + 66], 1.0)
                conv_group(5)

                # during q/k conv: transposes spread 1 per rb, pe on GpSimd
                nvt = [0]

                def vt_extra(rb):
                    if nvt[0] < 32:
                        make_vt(nvt[0])
                        nvt[0] += 1

                conv_group(0, extra=vt_extra)
                pe_conv(0, 0)
                pe_conv(0, 1)
                conv_group(1, extra=vt_extra)
                pe_conv(0, 2)
                pe_conv(0, 3)

                # pooling + block-diag a (both dtypes)
                for ccq in range(2):
                    qv = q_sb[ccq][:].rearrange(
                        "p (by dy bx dx) -> p by bx dy dx",
                        by=8, dy=8, bx=8, dx=8)
                    nc.vector.tensor_reduce(a_sum[ccq], qv, AX.XY, ALU.add)
                nc.vector.memset(abd_bf_t[:], 0.0)
                nc.vector.memset(abd_f_t[:].bitcast(f32), 0.0)
                for cc in range(2):
                    for j in range(4):
                        nc.vector.tensor_copy(
                            abd_bf[cc][32 * j:32 * j + 32,
                                       64 * j:64 * j + 64],
                            a_sum[cc][32 * j:32 * j + 32, :])
                        nc.vector.tensor_copy(
                            abd_f_t[32 * j:32 * j + 32,
                                    256 * cc + 64 * j:256 * cc + 64 * j + 64],
                            a_sum[cc][32 * j:32 * j + 32, :])

                # k
                conv_group(2, extra=vt_extra)
                pe_conv(1, 0)
                pe_conv(1, 1)
                conv_group(3, extra=vt_extra)
                pe_conv(1, 2)
                pe_conv(1, 3)
                while nvt[0] < 32:
                    make_vt(nvt[0])
                    nvt[0] += 1

            # ---- stage 1 ----
            # attn_ps[hp] accumulates [128 agents, 66] over 32 chunks:
            # for half 0 cols = [64 ch | Z Z], for half 1 cols = [Z Z | 64 ch]
            with ExitStack() as ph:
                st_ps = ph.enter_context(
                    tc.tile_pool(name="stps", bufs=3, space="PSUM"))
                at_ps = ph.enter_context(
                    tc.tile_pool(name="atps", bufs=4, space="PSUM"))
                etp = ph.enter_context(tc.tile_pool(name="etp", bufs=1))
                attn_ps = [at_ps.tile([128, 66], f32, tag="at", name="at")
                           for _ in range(4)]
                for chp in range(16):   # ch pairs
                    for cc in range(2):
                        sp = st_ps.tile([128, 512], f32, tag="st", name="stt")
                        for u in range(2):
                            ch = 2 * chp + u
                            nc.tensor.matmul(
                                sp[:, 256 * u:256 * (u + 1)],
                                k_sb[cc][:, 128 * ch:128 * (ch + 1)],
                                abd_bf[cc], start=True, stop=True,
                                skip_group_check=True)
                        et = etp.tile([128, 512], bf16, tag="et", name="et",
                                      bufs=4)
                        nc.scalar.activation(et[:], sp[:], AF.Exp, scale=SCALE)
                        for u in range(2):
                            ch = 2 * chp + u
                            for half in range(2):
                                hp = 2 * cc + half
                                rhs = vts[ch][:, 130 * cc + 64 * half:
                                              130 * cc + 64 * half + 66]
                                nc.tensor.matmul(
                                    attn_ps[hp][:],
                                    et[:, 256 * u + 128 * half:
                                       256 * u + 128 * (half + 1)],
                                    rhs, start=(ch == 0), stop=(ch == 31))

                # normalize stage-1 rows by Z1 -> attnZ [128 agents, 64]
                for hp in range(4):
                    half = hp % 2
                    zc = 64 if half == 0 else 0
                    och = 0 if half == 0 else 2
                    r1 = etp.tile([128, 1], f32, tag="r1", name="r1", bufs=4)
                    nc.vector.reciprocal(r1[:], attn_ps[hp][:, zc:zc + 1])
                    nc.vector.memset(attnZ[hp], 0.0)
                    nc.vector.tensor_scalar_mul(
                        attnZ[hp][0:64, 0:32],
                        attn_ps[hp][0:64, och:och + 32], r1[0:64, :])
                    nc.vector.tensor_scalar_mul(
                        attnZ[hp][64:128, 32:64],
                        attn_ps[hp][64:128, och + 32:och + 64],
                        r1[64:128, :])

            # ---- stage 2 + proj ----
            with ExitStack() as ph:
                s2sb = ph.enter_context(tc.tile_pool(name="s2sb", bufs=4))
                osb = ph.enter_context(tc.tile_pool(name="osb", bufs=3))
                s2_ps = ph.enter_context(
                    tc.tile_pool(name="s2ps", bufs=2, space="PSUM"))
                g_ps = ph.enter_context(
                    tc.tile_pool(name="gps", bufs=2, space="PSUM"))
                z_ps = ph.enter_context(
                    tc.tile_pool(name="zps", bufs=2, space="PSUM"))
                pr_ps = ph.enter_context(
                    tc.tile_pool(name="prps", bufs=2, space="PSUM"))

                for nt in range(8):
                    for cc in range(2):
                        gp = g_ps.tile([128, 512], f32, tag="g", name="gt")
                        zp = z_ps.tile([128, 512], f32, tag="z", name="zt")
                        for half in range(2):
                            hp = 2 * cc + half
                            sp = s2_ps.tile([128, 512], f32, tag="s2",
                                            name="s2t")
                            nc.tensor.matmul(
                                sp[:],
                                abd_f_t[:, 256 * cc + 128 * half:
                                        256 * cc + 128 * (half + 1)],
                                q_sb[cc][:, 512 * nt:512 * (nt + 1)],
                                start=True, stop=True)
                            e2 = s2sb.tile([128, 512], bf16, tag="e2",
                                           name="e2")
                            nc.scalar.activation(e2[:], sp[:], AF.Exp,
                                                 scale=SCALE)
                            # g rows 0:64 (half 0) / 64:128 (half 1)
                            nc.tensor.matmul(
                                gp[64 * half:64 * half + 64, :],
                                attnZ[hp], e2[:], start=True, stop=True,
                                skip_group_check=True)
                            # Zb rows: per-head agent sums of e2, already
                            # replicated to each head's 32 d-partitions
                            nc.tensor.matmul(
                                zp[64 * half:64 * half + 64, :],
                                hsel[:], e2[:], start=True, stop=True,
                                skip_group_check=True)
                        rb = s2sb.tile([128, 512], f32, tag="rb", name="rbt")
                        nc.vector.reciprocal_approx_fast(rb[:], zp[:])
                        tsc = s2sb.tile([128, 512], f32r, tag="ts",
                                        name="tsc")
                        nc.vector.tensor_tensor(tsc[:], gp[:], rb[:],
                                                ALU.mult)
                        sl = att_out[cc][:, 512 * nt:512 * (nt + 1)]
                        nc.vector.tensor_tensor(sl, tsc[:].bitcast(f32),
                                                sl.bitcast(f32), ALU.add)
                    for mc in range(2):
                        pp = pr_ps.tile([128, 512], f32, tag="tp", name="prt")
                        for kc in range(2):
                            nc.tensor.matmul(
                                pp[:], pwv[:, kc, 128 * mc:128 * (mc + 1)],
                                att_out[kc][:, 512 * nt:512 * (nt + 1)],
                                start=(kc == 0), stop=(kc == 1))
                        ot = osb.tile([128, 512], f32, tag="ot", name="ott")
                        nc.vector.tensor_scalar_add(ot[:], pp[:],
                                                    pb[:, mc:mc + 1])
                        nc.sync.dma_start(
                            OUT[mc, :, 512 * nt:512 * (nt + 1)], ot[:])

    nc.compile()
    return nc


def _prep_consts(qkv_w, qkv_s, qkv_b, pe_w, pe_s, pe_b, proj_w, proj_s,
                 proj_b):
    import ml_dtypes
    f = np.float32
    bf = ml_dtypes.bfloat16
    w = np.asarray(qkv_w, f).copy()          # [768, 256, 3, 3]
    dif = (w[:, :, 0, 1] + w[:, :, 1, 0] + w[:, :, 1, 1] + w[:, :, 1, 2]
           + w[:, :, 2, 1])
    w[:, :, 1, 1] -= THETA * dif
    w *= np.asarray(qkv_s, f)[:, None, None, None]
    # WQ[mc, p, kc, s, o'] = w[128*mc+o', 128*kc+p, s//3, s%3]
    wq = w.reshape(6, 128, 2, 128, 9)        # [mc, o', kc, p, s]
    wq = np.ascontiguousarray(wq.transpose(0, 3, 2, 4, 1))  # [6,128,2,9,128]

    bq = np.ascontiguousarray(np.asarray(qkv_b, f).reshape(6, 128).T)

    pe_wf = np.asarray(pe_w, f)[:, 0] * np.asarray(pe_s, f)[:, None, None]
    pew = np.zeros((128, 2, 9), f)
    for kc in range(2):
        for s in range(9):
            pew[:, kc, s] = pe_wf[128 * kc:128 * (kc + 1), s // 3, s % 3]

    pwm = np.asarray(proj_w, f)[:, :, 0, 0] * np.asarray(proj_s, f)[:, None]
    pw = np.ascontiguousarray(
        pwm.T.reshape(2, 128, 256).transpose(1, 0, 2).reshape(128, 512))
    pbv = np.asarray(proj_b, f) + pwm @ np.asarray(pe_b, f)
    pb = np.ascontiguousarray(pbv.reshape(2, 128).T)

    idn = np.eye(128, dtype=bf)
    return dict(wq=wq, bq=bq, pew=pew, pw=pw, pb=pb, idn=idn)


def kernel(x, qkv_w, qkv_s, qkv_b, pe_w, pe_s, pe_b, proj_w, proj_s, proj_b):
    import ml_dtypes
    from concourse.bass_utils import run_bass_kernel_spmd

    if "nc" not in _cache:
        _cache["nc"] = _build()
    nc = _cache["nc"]

    consts = _prep_consts(qkv_w, qkv_s, qkv_b, pe_w, pe_s, pe_b, proj_w,
                          proj_s, proj_b)
    x = np.asarray(x, np.float32).astype(ml_dtypes.bfloat16)
    in_maps = []
    for b in range(B):
        m = dict(consts)
        m["x"] = np.ascontiguousarray(x[b].reshape(2, 128, H, W))
        in_maps.append(m)

    res = run_bass_kernel_spmd(nc, in_maps, list(range(N_CORES)), trace=False)
    out = np.empty((B, C, H, W), np.float32)
    for b in range(B):
        out[b] = res.results[b]["out"].reshape(C, H, W)
    return out
